# revision 2
# baseline (speedup 1.0000x reference)
"""Householder reflection kernel for Trainium2 (8 NeuronCores, data-parallel).

Computes: v_new = v @ W.T + b
          z_new = z - 2 * v_new * (v_new . z) / ||v_new||^2

Full inputs: z [524288, 128] f32, v [524288, 128] f32, W [128, 128] f32, b [128] f32.
Sharding: batch split 8 ways; W/b replicated.

Device algorithm (per 128-row chunk), all fp32:
  - PE: transpose v -> PSUM; ACT copies to SBUF (vT)
  - PE: vnew = vT.T @ WT (+ rank-1 ones^T*b accumulate for the bias) -> PSUM
  - ACT: Square+accum on vnew -> norm ; DVE: tensor_tensor_reduce -> dot
  - GPSIMD: rn = 1/norm ; s = (dot * -2) * rn
  - DVE: z_new = (vnew * s) + z ; batched store
"""

import sys

if "/opt/trn_rl_repo" not in sys.path:
    sys.path.insert(0, "/opt/trn_rl_repo")

import numpy as np

B = 524288
D = 128
NCORES = 8
ROWS_PER_CORE = B // NCORES          # 65536
CHUNKS_PER_GROUP = 16                # 16 x 128 rows = 2048 rows per group
ROWS_PER_GROUP = CHUNKS_PER_GROUP * 128
GROUPS = ROWS_PER_CORE // ROWS_PER_GROUP  # 32

_compiled = None


def _build(rows_per_core=ROWS_PER_CORE):
    import concourse.bacc as bacc
    import concourse.tile as tile
    from concourse import mybir

    groups = rows_per_core // ROWS_PER_GROUP
    nc = bacc.Bacc("TRN2")
    f32 = mybir.dt.float32

    z_d = nc.dram_tensor("z", [rows_per_core, D], f32, kind="ExternalInput")
    v_d = nc.dram_tensor("v", [rows_per_core, D], f32, kind="ExternalInput")
    wt_d = nc.dram_tensor("wt", [D, D], f32, kind="ExternalInput")
    brow_d = nc.dram_tensor("brow", [1, D], f32, kind="ExternalInput")
    ident_d = nc.dram_tensor("ident", [128, 128], f32, kind="ExternalInput")
    out_d = nc.dram_tensor("z_new", [rows_per_core, D], f32, kind="ExternalOutput")

    # Group-tiled DRAM views: [group, partition(row%128), chunk, feature]
    zv = z_d.rearrange("(g k p) f -> g p k f", k=CHUNKS_PER_GROUP, p=128)
    vv = v_d.rearrange("(g k p) f -> g p k f", k=CHUNKS_PER_GROUP, p=128)
    ov = out_d.rearrange("(g k p) f -> g p k f", k=CHUNKS_PER_GROUP, p=128)

    with tile.TileContext(nc) as tc:
        import concourse.bass as bass
        from contextlib import ExitStack

        with ExitStack() as ctx:
            singles = ctx.enter_context(tc.tile_pool(name="singles", bufs=1))
            gpool = ctx.enter_context(tc.tile_pool(name="gpool", bufs=3))
            opool = ctx.enter_context(tc.tile_pool(name="opool", bufs=2))
            small = ctx.enter_context(tc.tile_pool(name="small", bufs=3))
            scr = ctx.enter_context(tc.tile_pool(name="scr", bufs=1))
            pt_pool = ctx.enter_context(tc.tile_pool(name="pt", bufs=2, space="PSUM"))
            pv_pool = ctx.enter_context(tc.tile_pool(name="pv", bufs=4, space="PSUM"))

            wt_sb = singles.tile([D, D], f32)
            nc.sync.dma_start(out=wt_sb, in_=wt_d.ap())
            brow_sb = singles.tile([1, D], f32)
            nc.sync.dma_start(out=brow_sb, in_=brow_d.ap())
            ones_sb = singles.tile([1, D], f32)
            nc.vector.memset(ones_sb, 1.0)
            ident_sb = singles.tile([128, 128], f32)
            nc.sync.dma_start(out=ident_sb, in_=ident_d.ap())

            # dead-write scratch targets
            prod_scr = scr.tile([128, D], f32)
            sq_scr = scr.tile([128, D], f32)

            for g in range(groups):
                vc_t = gpool.tile([128, CHUNKS_PER_GROUP, D], f32, tag="vc")
                z_t = gpool.tile([128, CHUNKS_PER_GROUP, D], f32, tag="z")
                zn_t = opool.tile([128, CHUNKS_PER_GROUP, D], f32, tag="zn")

                nc.sync.dma_start(out=vc_t, in_=vv[g])
                nc.sync.dma_start(out=z_t, in_=zv[g])

                for k in range(CHUNKS_PER_GROUP):
                    psum_t = pt_pool.tile([128, 128], f32, tag="pt")
                    nc.tensor.transpose(psum_t, vc_t[:, k, :], ident_sb)
                    vcT_sb = small.tile([128, 128], f32, tag="vcT")
                    nc.scalar.copy(out=vcT_sb, in_=psum_t)

                    psum_v = pv_pool.tile([128, D], f32, tag="pv")
                    nc.tensor.matmul(
                        psum_v, lhsT=vcT_sb, rhs=wt_sb, start=True, stop=False
                    )
                    nc.tensor.matmul(
                        psum_v, lhsT=ones_sb, rhs=brow_sb, start=False, stop=True
                    )

                    norm1 = small.tile([128, 1], f32, tag="norm")
                    nc.scalar.activation(
                        out=sq_scr,
                        in_=psum_v,
                        func=mybir.ActivationFunctionType.Square,
                        accum_out=norm1,
                    )
                    dot1 = small.tile([128, 1], f32, tag="dot")
                    nc.vector.scalar_tensor_tensor(
                        out=prod_scr,
                        in0=psum_v,
                        scalar=1.0,
                        in1=z_t[:, k, :],
                        op0=mybir.AluOpType.mult,
                        op1=mybir.AluOpType.mult,
                        accum_out=dot1,
                    )
                    rn1 = small.tile([128, 1], f32, tag="rn")
                    nc.vector.reciprocal(out=rn1, in_=norm1)
                    s1 = small.tile([128, 1], f32, tag="s")
                    nc.vector.tensor_scalar(
                        out=s1,
                        in0=dot1,
                        scalar1=rn1,
                        scalar2=-2.0,
                        op0=mybir.AluOpType.mult,
                        op1=mybir.AluOpType.mult,
                    )
                    nc.vector.scalar_tensor_tensor(
                        out=zn_t[:, k, :],
                        in0=psum_v,
                        scalar=s1,
                        in1=z_t[:, k, :],
                        op0=mybir.AluOpType.mult,
                        op1=mybir.AluOpType.add,
                    )

                nc.sync.dma_start(out=ov[g], in_=zn_t)

    nc.compile()
    return nc


def _get_compiled():
    global _compiled
    if _compiled is None:
        _compiled = _build()
    return _compiled


def kernel(z, v, W, b):
    from concourse.bass_utils import run_bass_kernel_spmd

    nc = _get_compiled()

    z = np.ascontiguousarray(z, dtype=np.float32)
    v = np.ascontiguousarray(v, dtype=np.float32)
    W64 = np.asarray(W, dtype=np.float64)
    b64 = np.asarray(b, dtype=np.float64)

    wt = np.ascontiguousarray(np.asarray(W, dtype=np.float32).T)
    brow = np.ascontiguousarray(np.asarray(b, dtype=np.float32).reshape(1, D))
    ident = np.eye(128, dtype=np.float32)

    in_maps = []
    for k in range(NCORES):
        sl = slice(k * ROWS_PER_CORE, (k + 1) * ROWS_PER_CORE)
        in_maps.append(
            {
                "z": z[sl],
                "v": v[sl],
                "wt": wt,
                "brow": brow,
                "ident": ident,
            }
        )

    res = run_bass_kernel_spmd(nc, in_maps, core_ids=list(range(NCORES)))
    global LAST_RESULT
    LAST_RESULT = res
    out = np.concatenate(
        [res.results[k]["z_new"] for k in range(NCORES)], axis=0
    )
    return out


LAST_RESULT = None



# revision 13
# speedup vs baseline: 1.3435x; 1.3435x over previous
"""Householder reflection kernel for Trainium2 (8 NeuronCores, data-parallel).

Computes: v_new = v @ W.T + b
          z_new = z - 2 * v_new * (v_new . z) / ||v_new||^2

Full inputs: z [524288, 128] f32, v [524288, 128] f32, W [128, 128] f32, b [128] f32.
Sharding: batch split 8 ways; W/b replicated. Memory-bound regime:
96 MB HBM traffic per core (~268 us floor at 358 GB/s).

Device algorithm (per core), bf16 compute path (rel-err ~2e-3 << 2e-2 budget):
  - v, z loaded via SWDGE cast-DMA (f32 HBM -> bf16 SBUF), 16 KB contiguous
    per partition per group (rows interleaved p-major so descriptors are big).
  - per supertile of 4 chunks (chunk = 128 rows x 128 feat):
      PE: 4x transpose(v chunk) -> one PSUM bank
      ACT: single [128,512] PSUM->SBUF copy-cast to bf16 (vT)
      PE: 4x matmul (lhsT=vT chunk, rhs=W^T bf16) + 1 bias matmul -> PSUM bank
      ACT: single [128,512] PSUM->SBUF copy-cast to bf16 (v_new)
      DVE: per chunk stt accum (norm, -2*dot) at 2x bf16 mode
      DVE: batched reciprocal + mult -> per-row scale s
      DVE: per chunk stt update z + s*v_new -> f32
  - store z_new f32 via HWDGE DMA.
"""

import sys

if "/opt/trn_rl_repo" not in sys.path:
    sys.path.insert(0, "/opt/trn_rl_repo")

import numpy as np

B = 524288
D = 128
NCORES = 8
ROWS_PER_CORE = B // NCORES          # 65536
CHUNKS_PER_GROUP = 32                # 32 x 128 rows = 4096 rows per group
ROWS_PER_GROUP = CHUNKS_PER_GROUP * 128
GROUPS = ROWS_PER_CORE // ROWS_PER_GROUP  # 16
SUPER = 4                            # chunks per PSUM supertile
SUPERS_PER_GROUP = CHUNKS_PER_GROUP // SUPER

_compiled = None


def _build(rows_per_core=ROWS_PER_CORE):
    import concourse.bacc as bacc
    import concourse.tile as tile
    from concourse import mybir

    groups = rows_per_core // ROWS_PER_GROUP
    nc = bacc.Bacc("TRN2")
    f32 = mybir.dt.float32
    bf16 = mybir.dt.bfloat16
    MUL = mybir.AluOpType.mult
    ADD = mybir.AluOpType.add
    DIV = mybir.AluOpType.divide

    z_d = nc.dram_tensor("z", [rows_per_core, D], f32, kind="ExternalInput")
    v_d = nc.dram_tensor("v", [rows_per_core, D], f32, kind="ExternalInput")
    wt_d = nc.dram_tensor("wt", [D, D], bf16, kind="ExternalInput")
    brow4_d = nc.dram_tensor("brow4", [1, SUPER, D], bf16, kind="ExternalInput")
    ident_d = nc.dram_tensor("ident", [128, 128], bf16, kind="ExternalInput")
    out_d = nc.dram_tensor("z_new", [rows_per_core, D], f32, kind="ExternalOutput")

    # Group-tiled DRAM views. Row index = (g*128 + p)*K + k so each
    # partition's slice of a group is K*512B contiguous bytes in DRAM.
    zv = z_d.rearrange("(g p k) f -> g p k f", p=128, k=CHUNKS_PER_GROUP)
    vv = v_d.rearrange("(g p k) f -> g p k f", p=128, k=CHUNKS_PER_GROUP)
    ov = out_d.rearrange("(g p k) f -> g p k f", p=128, k=CHUNKS_PER_GROUP)

    with tile.TileContext(nc) as tc:
        from contextlib import ExitStack

        with ExitStack() as ctx:
            singles = ctx.enter_context(tc.tile_pool(name="singles", bufs=1))
            vzpool = ctx.enter_context(tc.tile_pool(name="vz", bufs=3))
            opool = ctx.enter_context(tc.tile_pool(name="op", bufs=2))
            vtpool = ctx.enter_context(tc.tile_pool(name="vt", bufs=3))
            small = ctx.enter_context(tc.tile_pool(name="small", bufs=4))
            pt_pool = ctx.enter_context(tc.tile_pool(name="pt", bufs=2, space="PSUM"))
            pv_pool = ctx.enter_context(tc.tile_pool(name="pv", bufs=3, space="PSUM"))

            wt_sb = singles.tile([D, D], bf16)
            nc.sync.dma_start(out=wt_sb, in_=wt_d.ap())
            brow4_sb = singles.tile([1, SUPER, D], bf16)
            nc.sync.dma_start(out=brow4_sb, in_=brow4_d.ap())
            ident_sb = singles.tile([128, 128], bf16)
            nc.sync.dma_start(out=ident_sb, in_=ident_d.ap())
            ones_sb = singles.tile([1, D], bf16)
            nc.vector.memset(ones_sb, 1.0)

            # dead-write scratch targets for accumulating ops
            scr_f = singles.tile([128, D], f32)
            sq_pool = ctx.enter_context(tc.tile_pool(name="sq", bufs=1, space="PSUM"))
            sq_scr = sq_pool.tile([128, D], f32)

            for g in range(groups):
                v_bf = vzpool.tile([128, CHUNKS_PER_GROUP, D], bf16, tag="v")
                z_bf = vzpool.tile([128, CHUNKS_PER_GROUP, D], bf16, tag="z")
                zn_t = opool.tile([128, CHUNKS_PER_GROUP, D], f32, tag="zn")

                # SWDGE cast-DMA loads: f32 HBM -> bf16 SBUF
                nc.gpsimd.dma_start(out=v_bf, in_=vv[g])
                nc.gpsimd.dma_start(out=z_bf, in_=zv[g])

                for s in range(SUPERS_PER_GROUP):
                    pt = pt_pool.tile([128, SUPER, 128], bf16, tag="pt")
                    for c in range(SUPER):
                        nc.tensor.transpose(
                            pt[:, c, :], v_bf[:, s * SUPER + c, :], ident_sb
                        )
                    vT = vtpool.tile([128, SUPER, 128], bf16, tag="vt")
                    nc.scalar.copy(out=vT, in_=pt)

                    pv = pv_pool.tile([128, SUPER, D], f32, tag="pv")
                    for c in range(SUPER):
                        # NOTE: the bias matmul must come right after its
                        # region's main matmul — a single deferred bias matmul
                        # over all 4 regions overwrites (has_written granularity).
                        nc.tensor.matmul(
                            pv[:, c, :],
                            lhsT=vT[:, c, :],
                            rhs=wt_sb,
                            start=True,
                            stop=False,
                        )
                        nc.tensor.matmul(
                            pv[:, c, :],
                            lhsT=ones_sb,
                            rhs=brow4_sb[:, 0, :],
                            start=False,
                            stop=True,
                        )
                    # nd cols 0..3 = ||v_new||^2 (ACT), cols 4..7 = -2*(v_new . z) (DVE)
                    nd = small.tile([128, 2 * SUPER], f32, tag="nd")
                    for c in range(SUPER):
                        nc.scalar.activation(
                            out=sq_scr,
                            in_=pv[:, c, :],
                            func=mybir.ActivationFunctionType.Square,
                            accum_out=nd[:, c : c + 1],
                        )
                        nc.vector.scalar_tensor_tensor(
                            out=scr_f,
                            in0=pv[:, c, :],
                            scalar=-2.0,
                            in1=z_bf[:, s * SUPER + c, :],
                            op0=MUL,
                            op1=MUL,
                            accum_out=nd[:, SUPER + c : SUPER + c + 1],
                        )
                    rn = small.tile([128, SUPER], f32, tag="rn")
                    nc.vector.reciprocal(out=rn, in_=nd[:, 0:SUPER])
                    s4 = small.tile([128, SUPER], f32, tag="s4")
                    nc.vector.tensor_tensor(
                        out=s4, in0=nd[:, SUPER : 2 * SUPER], in1=rn, op=MUL
                    )
                    for c in range(SUPER):
                        nc.vector.scalar_tensor_tensor(
                            out=zn_t[:, s * SUPER + c, :],
                            in0=pv[:, c, :],
                            scalar=s4[:, c : c + 1],
                            in1=z_bf[:, s * SUPER + c, :],
                            op0=MUL,
                            op1=ADD,
                        )

                nc.sync.dma_start(out=ov[g], in_=zn_t)

    nc.compile()
    return nc


def _get_compiled():
    global _compiled
    if _compiled is None:
        _compiled = _build()
    return _compiled


def kernel(z, v, W, b):
    import ml_dtypes
    from concourse.bass_utils import run_bass_kernel_spmd

    nc = _get_compiled()
    bf16 = ml_dtypes.bfloat16

    z = np.ascontiguousarray(z, dtype=np.float32)
    v = np.ascontiguousarray(v, dtype=np.float32)

    wt = np.ascontiguousarray(np.asarray(W, dtype=np.float32).T.astype(bf16))
    brow4 = np.ascontiguousarray(
        np.tile(np.asarray(b, dtype=np.float32).astype(bf16).reshape(1, 1, D),
                (1, SUPER, 1))
    )
    ident = np.eye(128, dtype=bf16)

    in_maps = []
    for k in range(NCORES):
        sl = slice(k * ROWS_PER_CORE, (k + 1) * ROWS_PER_CORE)
        in_maps.append(
            {
                "z": z[sl],
                "v": v[sl],
                "wt": wt,
                "brow4": brow4,
                "ident": ident,
            }
        )

    res = run_bass_kernel_spmd(nc, in_maps, core_ids=list(range(NCORES)))
    global LAST_RESULT
    LAST_RESULT = res
    out = np.concatenate(
        [res.results[k]["z_new"] for k in range(NCORES)], axis=0
    )
    return out


LAST_RESULT = None


# revision 14
# speedup vs baseline: 1.8846x; 1.4028x over previous
"""Householder reflection kernel for Trainium2 (8 NeuronCores, data-parallel).

Computes: v_new = v @ W.T + b
          z_new = z - 2 * v_new * (v_new . z) / ||v_new||^2

Full inputs: z [524288, 128] f32, v [524288, 128] f32, W [128, 128] f32, b [128] f32.
Sharding: batch split 8 ways; W/b replicated. Memory-bound regime:
96 MB HBM traffic per core (~268 us floor at 358 GB/s).

Per-core pipeline (bf16 compute, rel-err ~2e-3 << 2e-2 budget), organized to
minimize per-op fixed costs (wide ops, no accumulator reads):
  loads: SWDGE cast-DMA f32 HBM -> bf16 SBUF, 16 KB/partition contiguous.
  per supertile of 4 chunks (chunk = 128 rows x 128 feat):
    PE : 4x transpose(v_c) -> PSUM (bf16)
    ACT: one [128,512] copy-cast PSUM -> SBUF (vT)
    PE : 4x matmul (lhsT=vT_c, rhs=W^T) start only on c==0, one trailing
         bias matmul over the whole [128,512] tile (accumulates via has_written)
    ACT: one [128,512] copy-cast v_new -> SBUF bf16 (vn)
    ACT: one [128,512] Square(sqrt(.5)*pv) -> SP[:,0:4,:]   (= .5*vn^2)
    DVE: one wide TT vn*z -> SP[:,4:8,:]
    DVE: one segmented reduce SP [128,8,128] -> nd [128,8]
         (nd[:,0:4] = .5*norm, nd[:,4:8] = dot)
    DVE: recip + mult -> s4 = 2*dot/norm
    GPS: 4x tensor_scalar t_c = vn_c * s4_c * (-1)
    GPS: one wide TT add zn = t + z (f32 out)
  store: z_new f32 via HWDGE DMA.
"""

import sys

if "/opt/trn_rl_repo" not in sys.path:
    sys.path.insert(0, "/opt/trn_rl_repo")

import numpy as np

B = 524288
D = 128
NCORES = 8
ROWS_PER_CORE = B // NCORES          # 65536
CHUNKS_PER_GROUP = 32                # 32 x 128 rows = 4096 rows per group
ROWS_PER_GROUP = CHUNKS_PER_GROUP * 128
GROUPS = ROWS_PER_CORE // ROWS_PER_GROUP  # 16
SUPER = 4                            # chunks per PSUM supertile
SUPERS_PER_GROUP = CHUNKS_PER_GROUP // SUPER

_compiled = None


def _build(rows_per_core=ROWS_PER_CORE):
    import concourse.bacc as bacc
    import concourse.tile as tile
    from concourse import mybir

    groups = rows_per_core // ROWS_PER_GROUP
    nc = bacc.Bacc("TRN2")
    f32 = mybir.dt.float32
    bf16 = mybir.dt.bfloat16
    MUL = mybir.AluOpType.mult
    ADD = mybir.AluOpType.add

    z_d = nc.dram_tensor("z", [rows_per_core, D], f32, kind="ExternalInput")
    v_d = nc.dram_tensor("v", [rows_per_core, D], f32, kind="ExternalInput")
    wt_d = nc.dram_tensor("wt", [D, D], bf16, kind="ExternalInput")
    brow4_d = nc.dram_tensor("brow4", [1, SUPER * D], bf16, kind="ExternalInput")
    ident_d = nc.dram_tensor("ident", [128, 128], bf16, kind="ExternalInput")
    out_d = nc.dram_tensor("z_new", [rows_per_core, D], f32, kind="ExternalOutput")

    # Group-tiled DRAM views. Row index = (g*128 + p)*K + k so each
    # partition's slice of a group is K*512B contiguous bytes in DRAM.
    zv = z_d.rearrange("(g p k) f -> g p k f", p=128, k=CHUNKS_PER_GROUP)
    vv = v_d.rearrange("(g p k) f -> g p k f", p=128, k=CHUNKS_PER_GROUP)
    ov = out_d.rearrange("(g p k) f -> g p k f", p=128, k=CHUNKS_PER_GROUP)

    SQRT_HALF = float(np.sqrt(0.5))

    with tile.TileContext(nc) as tc:
        from contextlib import ExitStack

        with ExitStack() as ctx:
            singles = ctx.enter_context(tc.tile_pool(name="singles", bufs=1))
            vzpool = ctx.enter_context(tc.tile_pool(name="vz", bufs=3))
            opool = ctx.enter_context(tc.tile_pool(name="op", bufs=2))
            vtpool = ctx.enter_context(tc.tile_pool(name="vt", bufs=3))
            vnpool = ctx.enter_context(tc.tile_pool(name="vn", bufs=3))
            sppool = ctx.enter_context(tc.tile_pool(name="sp", bufs=3))
            tpool = ctx.enter_context(tc.tile_pool(name="tp", bufs=3))
            small = ctx.enter_context(tc.tile_pool(name="small", bufs=4))
            pt_pool = ctx.enter_context(tc.tile_pool(name="pt", bufs=2, space="PSUM"))
            pv_pool = ctx.enter_context(tc.tile_pool(name="pv", bufs=3, space="PSUM"))

            wt_sb = singles.tile([D, D], bf16)
            nc.sync.dma_start(out=wt_sb, in_=wt_d.ap())
            brow4_sb = singles.tile([1, SUPER * D], bf16)
            nc.sync.dma_start(out=brow4_sb, in_=brow4_d.ap())
            ident_sb = singles.tile([128, 128], bf16)
            nc.sync.dma_start(out=ident_sb, in_=ident_d.ap())
            ones_sb = singles.tile([1, D], bf16)
            nc.vector.memset(ones_sb, 1.0)

            for g in range(groups):
                v_bf = vzpool.tile([128, CHUNKS_PER_GROUP, D], bf16, tag="v")
                z_bf = vzpool.tile([128, CHUNKS_PER_GROUP, D], bf16, tag="z")
                zn_t = opool.tile([128, CHUNKS_PER_GROUP, D], f32, tag="zn")

                # SWDGE cast-DMA loads: f32 HBM -> bf16 SBUF
                nc.gpsimd.dma_start(out=v_bf, in_=vv[g])
                nc.gpsimd.dma_start(out=z_bf, in_=zv[g])

                for s in range(SUPERS_PER_GROUP):
                    pt = pt_pool.tile([128, SUPER, 128], bf16, tag="pt")
                    for c in range(SUPER):
                        nc.tensor.transpose(
                            pt[:, c, :], v_bf[:, s * SUPER + c, :], ident_sb
                        )
                    vT = vtpool.tile([128, SUPER, 128], bf16, tag="vt")
                    nc.scalar.copy(out=vT, in_=pt)

                    pv = pv_pool.tile([128, SUPER, D], f32, tag="pv")
                    for c in range(SUPER):
                        nc.tensor.matmul(
                            pv[:, c, :],
                            lhsT=vT[:, c, :],
                            rhs=wt_sb,
                            start=(c == 0),
                            stop=False,
                        )
                    # one bias matmul for the whole supertile; accumulates onto
                    # all 4 regions (their has_written bits are set)
                    nc.tensor.matmul(
                        pv, lhsT=ones_sb, rhs=brow4_sb, start=False, stop=True
                    )

                    vn = vnpool.tile([128, SUPER, D], bf16, tag="vn")
                    nc.scalar.copy(out=vn, in_=pv)

                    # SP[:,0:4,:] = 0.5*vn^2 ; SP[:,4:8,:] = vn*z
                    sp = sppool.tile([128, 2 * SUPER, D], bf16, tag="sp")
                    nc.scalar.activation(
                        out=sp[:, 0:SUPER, :],
                        in_=pv,
                        func=mybir.ActivationFunctionType.Square,
                        scale=SQRT_HALF,
                    )
                    nc.vector.tensor_tensor(
                        out=sp[:, SUPER : 2 * SUPER, :],
                        in0=vn,
                        in1=z_bf[:, s * SUPER : (s + 1) * SUPER, :],
                        op=MUL,
                    )
                    # nd[:,0:4] = 0.5*norm ; nd[:,4:8] = dot
                    nd = small.tile([128, 2 * SUPER], f32, tag="nd")
                    nc.vector.tensor_reduce(
                        out=nd, in_=sp, op=ADD, axis=mybir.AxisListType.X
                    )
                    rn = small.tile([128, SUPER], f32, tag="rn")
                    nc.vector.reciprocal(out=rn, in_=nd[:, 0:SUPER])
                    s4 = small.tile([128, SUPER], f32, tag="s4")
                    nc.vector.tensor_tensor(
                        out=s4, in0=nd[:, SUPER : 2 * SUPER], in1=rn, op=MUL
                    )

                    # t_c = vn_c * s4_c * (-1)  (GPSIMD), then zn = t + z
                    t_t = tpool.tile([128, SUPER, D], bf16, tag="t")
                    for c in range(SUPER):
                        nc.gpsimd.tensor_scalar(
                            out=t_t[:, c, :],
                            in0=vn[:, c, :],
                            scalar1=s4[:, c : c + 1],
                            scalar2=-1.0,
                            op0=MUL,
                            op1=MUL,
                        )
                    nc.gpsimd.tensor_tensor(
                        out=zn_t[:, s * SUPER : (s + 1) * SUPER, :],
                        in0=t_t,
                        in1=z_bf[:, s * SUPER : (s + 1) * SUPER, :],
                        op=ADD,
                    )

                nc.sync.dma_start(out=ov[g], in_=zn_t)

    nc.compile()
    return nc


def _get_compiled():
    global _compiled
    if _compiled is None:
        _compiled = _build()
    return _compiled


def kernel(z, v, W, b):
    import ml_dtypes
    from concourse.bass_utils import run_bass_kernel_spmd

    nc = _get_compiled()
    bf16 = ml_dtypes.bfloat16

    z = np.ascontiguousarray(z, dtype=np.float32)
    v = np.ascontiguousarray(v, dtype=np.float32)

    wt = np.ascontiguousarray(np.asarray(W, dtype=np.float32).T.astype(bf16))
    brow4 = np.ascontiguousarray(
        np.tile(np.asarray(b, dtype=np.float32).astype(bf16).reshape(1, D),
                (1, SUPER))
    )
    ident = np.eye(128, dtype=bf16)

    in_maps = []
    for k in range(NCORES):
        sl = slice(k * ROWS_PER_CORE, (k + 1) * ROWS_PER_CORE)
        in_maps.append(
            {
                "z": z[sl],
                "v": v[sl],
                "wt": wt,
                "brow4": brow4,
                "ident": ident,
            }
        )

    res = run_bass_kernel_spmd(nc, in_maps, core_ids=list(range(NCORES)))
    global LAST_RESULT
    LAST_RESULT = res
    out = np.concatenate(
        [res.results[k]["z_new"] for k in range(NCORES)], axis=0
    )
    return out


LAST_RESULT = None
